# revision 6
# baseline (speedup 1.0000x reference)
"""AttnBlock (GroupNorm -> QKV 1x1 -> single-head attention over 4096 tokens
-> out 1x1 -> residual) for B=4, C=512, H=W=64 on 8 trn2 NeuronCores.

Sharding: core m handles sample m//2 and query tokens [0:2048] of a
token-rotated copy of the sample (softmax over keys is permutation
invariant; GroupNorm stats are position invariant).

The whole GEMM chain runs in fp8e4m3 DoubleRow perf mode (2 fp8 weights
per PE cell + 2 output columns per cycle = 4x bf16 matmul throughput).
Channel pairs are interleaved [128, 2, n] so no on-device transpose is
ever needed:
  xn2[cs]  [128, 2, t]  fp8   GN output, cin = cs*256 + s*128 + p
  K2[cs]   [128, 2, t]  fp8   = wk2.T @ xn2        (bias dropped: (q+bq)@bk
                               is j-independent, softmax kills it)
  Q2[cs]   [128, 2, i]  fp8   = wq2.T @ xn2[:2048] + bq  (ACT Identity)
  V^T      [t, 2, c]    fp8   = xn2.T @ wv2   (bv host-folded into bo via
                               bo' = bo + wo@bv: softmax rows sum to 1)
  S^T      [j, i]  PSUM f32   = K2.T @ Q2     (contraction c = 512)
  P^T = exp(S^T*SCALE/64 - 2.25)  fp8  (scores ~N(0,1) by construction:
                               no max subtraction; +2.25 shift cancels)
  sums[1,i] = ones.T @ P^T, O[c,i] = (V^T).T @ P^T   (DR accumulation)
  o2 = O/128 in fp8 pairs (unnormalized O*8 reaches ~3500, past fp8's
       448 max; /128 keeps the tail in range)
  y   [o, i] = wo2.T @ o2  (fp8 DR), then y*(128/64)*recip(sums) + bo + x

wq/wk/wv/wo are scaled by 8 on the host so w ~ N(0, 1/512) lands in
fp8e4m3's normal range; the scale powers fold into the exp input scale
(1/64) and the softmax reciprocal (128/64).

Hardware rules this schedule is built around (all verified empirically):
 - fp8 DoubleRow matmuls must NOT interleave instruction-by-instruction
   with bf16/f32 matmuls (corrupts on hw): all f32 GroupNorm/warmup
   matmuls strictly precede fp8-land; everything after is fp8 DR.
 - Two CONCURRENT PSUM accumulation groups in one bank corrupt each
   other; concurrent groups must sit in distinct banks (serial reuse is
   fine).  Hence O accumulates as pass A (sums + channel blocks 0-2 in
   the su + 3 oa banks) and pass B (block 3 reusing the su bank after
   the reciprocal read sums), re-reading P^T from SBUF.
 - GPSIMD cannot read PSUM, and its SWDGE DMAs cost ~1us each: Pool gets
   only SBUF->SBUF work (GN apply share, mid-stream y finals) and no
   DMA triggers.
 - PSUM rings are split by draining engine (ps_st: ACT exp/Identity;
   ps_kv: DVE copies) so one engine's backlog never stalls the other's
   ring rotation; P^T rotates 3 SBUF phase sets because pass B of chunk
   ic still reads set ic%3 while chunk ic+2 writes.

Steady-state schedule (ACT exp stream ~98us is the pacer, PE ~83us):
chunk ic's S^T/exp windows carry consume-pass-A of ic-1, pass-B of ic-2,
projection/V fillers, and the fp8 y-conv of ic-2; per-chunk reciprocal
broadcasts use Pool partition_broadcast (SBUF->SBUF).  bn_stats is
DVE-only and serial (~19us): x tiles stream in quarters/halves over both
HWDGE queues so stats start ~2.7us in, with the GN apply split ACT/Pool.
"""

import threading

import numpy as np
import ml_dtypes

import concourse.bacc as bacc
import concourse.tile as tile
import concourse.mybir as mybir

F32 = mybir.dt.float32
BF16 = mybir.dt.bfloat16
FP8 = mybir.dt.float8e4
DR = mybir.MatmulPerfMode.DoubleRow
AF = mybir.ActivationFunctionType
OP = mybir.AluOpType
SCALE = 1.0 / (512.0 ** 0.5)
WS = 8.0            # host weight scale for wq/wk/wv/wo
OS = 128.0          # o2 evac divisor: unnormalized O*WS reaches ~3500*8,
                    # far past fp8e4m3's 448 max; /128 keeps the tail <256

B, C, H, W = 4, 512, 64, 64
HW = H * W          # 4096
HALF = HW // 2      # 2048 query tokens per core
GROUPS = 32         # 16 channels per group -> 8 groups per 128-partition tile
EPS = 1e-6
NCORES = 8
CT = C // 128       # 4 channel tiles
CS = 2              # channel superblocks of 256 (fp8 pair-packed)
JB = HW // 128      # 32 key blocks
IC = HALF // 512    # 4 query chunks
JC = HW // 512      # 8 token chunks
NP = JB // 2        # 16 token-pair super-blocks (contraction 256 each)


def build_bass():
    nc = bacc.Bacc("TRN2", target_bir_lowering=False, debug=False,
                   num_devices=NCORES)

    xbf = nc.dram_tensor("xbf", [C, HW], BF16, kind="ExternalInput").ap()
    xres = nc.dram_tensor("xres", [C, HALF], F32, kind="ExternalInput").ap()
    # fp8 pair-packed weights [128, cs, s, 512]: w2[p, cs, s, o] =
    # w[o, cs*256 + s*128 + p] * WS
    wq2d = nc.dram_tensor("wq2", [128, CS * 2 * C], FP8,
                          kind="ExternalInput").ap()
    wk2d = nc.dram_tensor("wk2", [128, CS * 2 * C], FP8,
                          kind="ExternalInput").ap()
    wv2d = nc.dram_tensor("wv2", [128, CS * 2 * C], FP8,
                          kind="ExternalInput").ap()
    wo2d = nc.dram_tensor("wo2", [128, CS * 2 * C], FP8,
                          kind="ExternalInput").ap()
    # per-channel scalars packed [128, ct*5 + {bq*WS, bk*WS, bo, gnw, gnb}]
    colb = nc.dram_tensor("colb", [128, CT * 5], F32,
                          kind="ExternalInput").ap()
    gmap = nc.dram_tensor("gmap", [128, 128], F32, kind="ExternalInput").ap()
    y = nc.dram_tensor("y", [C, HALF], F32, kind="ExternalOutput").ap()

    with tile.TileContext(nc) as tc:
        # ---- persistent pools ----
        consts = tc.alloc_tile_pool(name="consts", bufs=1)
        wpool = tc.alloc_tile_pool(name="wpool", bufs=1)
        kpool = tc.alloc_tile_pool(name="kpool", bufs=1)
        vpool = tc.alloc_tile_pool(name="vpool", bufs=1)
        qpool = tc.alloc_tile_pool(name="qpool", bufs=1)
        ptpool = tc.alloc_tile_pool(name="ptpool", bufs=1)

        eps_t = consts.tile([128, 1], F32, name="eps_t")
        nc.vector.memset(eps_t, EPS)
        negs_t = consts.tile([128, 1], F32, name="negs_t")
        nc.vector.memset(negs_t, -2.25)
        # dummy activations: ACT table loads happen during startup stats
        warm_t = consts.tile([128, 1], F32, name="warm_t")
        nc.scalar.activation(out=warm_t, in_=eps_t, func=AF.Exp)
        nc.scalar.activation(out=warm_t, in_=eps_t, func=AF.Sqrt)
        nc.scalar.activation(out=warm_t, in_=eps_t, func=AF.Identity,
                             bias=negs_t)

        w2 = {nm: wpool.tile([128, CS, 2, C], FP8, name=f"w2{nm}")
              for nm in ("q", "k", "v", "o")}

        # K/Q fp8 pair-packed over channels: k2[cs][p, s, j]
        k2_t = [kpool.tile([128, 2, HW], FP8, name=f"k2_{cs}")
                for cs in range(CS)]
        q2_t = [qpool.tile([128, 2, HALF], FP8, name=f"q2_{cs}")
                for cs in range(CS)]
        # V^T fp8 token pairs: vt2[jp][p, s, c] = V^T[jp*256 + s*128 + p, c]
        vt2_t = [vpool.tile([128, 2, C], FP8, name=f"vt2_{jp}")
                 for jp in range(NP)]
        # P^T triple-buffered across chunks (pt_t[ic%3][jp]): chunk ic's
        # exps write set ic%3 while pass A of ic-1 reads set (ic-1)%3 and
        # pass B of ic-2 reads set (ic-2)%3 -- with only 2 sets the pass-B
        # reads serialize against the next chunk's exps
        pt_t = [[ptpool.tile([128, 2, 512], FP8, name=f"pt{ph}_{jp}")
                 for jp in range(NP)] for ph in range(3)]

        # ================= phase 1: GroupNorm =================
        xnpool = tc.alloc_tile_pool(name="xnpool", bufs=1)
        xfpool = tc.alloc_tile_pool(name="xfpool", bufs=1)
        stpool = tc.alloc_tile_pool(name="stpool", bufs=4)
        ps_sg = tc.alloc_tile_pool(name="ps_sg", bufs=2, space="PSUM")

        # keep the PE p-state window alive through the startup stats chain
        # (f32 dummies: strictly before any fp8 DR matmul)
        def pe_warm(n):
            for _ in range(n):
                wps = ps_sg.tile([1, 1], F32, name="wps", tag="gs")
                nc.tensor.matmul(wps, eps_t, eps_t, start=True, stop=True)

        xf_tiles = [xfpool.tile([128, HW], BF16, name="xf_t", tag=f"xf{ct}")
                    for ct in range(CT)]
        gmap_t = consts.tile([128, 128], F32, name="gmap_t")
        colb_t = consts.tile([128, CT * 5], F32, name="colb_t")
        # x tiles split in halves across the two HWDGE queues so bn_stats
        # of tile ct can start ~2.7us after launch; no gpsimd (SWDGE) DMAs
        # anywhere (994ns Q7 descriptor-gen each)
        Q4 = HALF // 2
        nc.sync.dma_start(out=xf_tiles[0][:, :Q4], in_=xbf[0:128, :Q4])
        nc.scalar.dma_start(out=xf_tiles[0][:, HALF:HALF + Q4],
                            in_=xbf[0:128, HALF:HALF + Q4])
        nc.sync.dma_start(out=xf_tiles[0][:, Q4:HALF], in_=xbf[0:128, Q4:HALF])
        nc.scalar.dma_start(out=xf_tiles[0][:, HALF + Q4:],
                            in_=xbf[0:128, HALF + Q4:])
        nc.scalar.dma_start(out=gmap_t, in_=gmap)
        nc.scalar.dma_start(out=colb_t, in_=colb)
        for ct in range(1, CT):
            csl = slice(ct * 128, (ct + 1) * 128)
            nc.sync.dma_start(out=xf_tiles[ct][:, :HALF],
                              in_=xbf[csl, :HALF])
            nc.scalar.dma_start(out=xf_tiles[ct][:, HALF:],
                                in_=xbf[csl, HALF:])
        nc.sync.dma_start(out=w2["k"], in_=wk2d)
        nc.sync.dma_start(out=w2["v"], in_=wv2d)
        nc.scalar.dma_start(out=w2["q"], in_=wq2d)
        nc.scalar.dma_start(out=w2["o"], in_=wo2d)

        bqs_t = [colb_t[:, ct * 5 + 0:ct * 5 + 1] for ct in range(CT)]
        bk_t = [colb_t[:, ct * 5 + 1:ct * 5 + 2] for ct in range(CT)]
        bo_t = [colb_t[:, ct * 5 + 2:ct * 5 + 3] for ct in range(CT)]
        gnw_t = [colb_t[:, ct * 5 + 3:ct * 5 + 4] for ct in range(CT)]
        gnb_t = [colb_t[:, ct * 5 + 4:ct * 5 + 5] for ct in range(CT)]

        # xn2[cs][p, s, t] fp8: cin = cs*256 + s*128 + p
        xn2_t = [xnpool.tile([128, 2, HW], FP8, name=f"xn2_{cs}")
                 for cs in range(CS)]

        pe_warm(10)

        def do_stats(ct):
            stats = stpool.tile([128, 8, 6], F32, name="stats", tag="stats")
            for s in range(8):
                nc.vector.bn_stats(out=stats[:, s, :],
                                   in_=xf_tiles[ct][:, s * 512:(s + 1) * 512])
            mv = stpool.tile([128, 2], F32, name="mv", tag="mv")
            nc.vector.bn_aggr(out=mv, in_=stats)
            return mv

        def do_chain(ct, mv):
            # rhs2 = [mean, E[x^2]] per channel
            rhs2 = stpool.tile([128, 2], F32, name="rhs2", tag="rhs2")
            nc.vector.tensor_copy(out=rhs2[:, 0:1], in_=mv[:, 0:1])
            nc.vector.scalar_tensor_tensor(
                out=rhs2[:, 1:2], in0=mv[:, 0:1], scalar=1.0, in1=mv[:, 0:1],
                op0=OP.mult, op1=OP.mult)
            nc.vector.tensor_add(out=rhs2[:, 1:2], in0=rhs2[:, 1:2],
                                 in1=mv[:, 1:2])
            gs_ps = ps_sg.tile([128, 2], F32, name="gs_ps", tag="gs")
            nc.tensor.matmul(gs_ps, gmap_t, rhs2, start=True, stop=True)
            gs = stpool.tile([128, 2], F32, name="gs", tag="gs")
            nc.scalar.copy(out=gs, in_=gs_ps)
            # A = gnw * rsqrt(var+eps); Bc = gnb - mu*A
            var_t = stpool.tile([128, 1], F32, name="var_t", tag="var")
            nc.vector.scalar_tensor_tensor(
                out=var_t, in0=gs[:, 0:1], scalar=-1.0, in1=gs[:, 0:1],
                op0=OP.mult, op1=OP.mult)
            nc.vector.tensor_add(out=var_t, in0=var_t, in1=gs[:, 1:2])
            nc.scalar.activation(out=var_t, in_=var_t, func=AF.Sqrt,
                                 bias=eps_t)
            nc.vector.reciprocal(out=var_t, in_=var_t)
            a_t = stpool.tile([128, 1], F32, name="a_t", tag="a")
            nc.vector.tensor_mul(out=a_t, in0=var_t, in1=gnw_t[ct])
            b_t = stpool.tile([128, 1], F32, name="b_t", tag="b")
            nc.vector.scalar_tensor_tensor(
                out=b_t, in0=gs[:, 0:1], scalar=-1.0, in1=a_t,
                op0=OP.mult, op1=OP.mult)
            nc.vector.tensor_add(out=b_t, in0=b_t, in1=gnb_t[ct])
            # apply-pass split ACT/Pool: Identity(x*a + b) into fp8 pairs
            xn_sl = xn2_t[ct // 2][:, ct % 2, :]
            act_share = 5 if ct == CT - 1 else 2
            for jc in range(JC):
                sl = slice(jc * 512, (jc + 1) * 512)
                if jc < act_share:
                    nc.scalar.activation(out=xn_sl[:, sl],
                                         in_=xf_tiles[ct][:, sl],
                                         func=AF.Identity, bias=b_t,
                                         scale=a_t)
                else:
                    nc.gpsimd.tensor_scalar(out=xn_sl[:, sl],
                                            in0=xf_tiles[ct][:, sl],
                                            scalar1=a_t, scalar2=b_t,
                                            op0=OP.mult, op1=OP.add)

        for ct in range(CT):
            do_chain(ct, do_stats(ct))
            pe_warm(6)

        ps_sg.release()  # f32 matmuls done; fp8-land below
        stpool.release()
        xfpool.release()

        # ================= phase 2: fp8 DR land =================
        xrpool = tc.alloc_tile_pool(name="xrpool", bufs=1)
        opool = tc.alloc_tile_pool(name="opool", bufs=2)
        finpool = tc.alloc_tile_pool(name="finpool", bufs=2)
        # PSUM rings split by evacuating engine so a slow evac on one
        # engine never blocks the other engine's pipeline:
        #   ps_st (2): S^T + Q singles, always drained by ACT (exp/Identity)
        #   ps_kv (2): K/V/y singles, always drained by DVE
        #   ps_oa (3): O accumulators for channel blocks 0-2 (pass A)
        #   ps_su (1): sums during pass A, then REUSED as the cb3 O
        #   accumulator (pass B re-reads pt) once the reciprocal has
        #   consumed sums -- concurrent matmul accumulation groups must
        #   live in DISTINCT PSUM banks (same-bank concurrency corrupts),
        #   so the bank is time-shared via the same pool tag.
        ps_st = tc.alloc_tile_pool(name="ps_st", bufs=2, space="PSUM")
        ps_kv = tc.alloc_tile_pool(name="ps_kv", bufs=2, space="PSUM")
        ps_su = tc.alloc_tile_pool(name="ps_su", bufs=1, space="PSUM")
        ps_oa = tc.alloc_tile_pool(name="ps_oa", bufs=1, space="PSUM")

        xr_t = []
        for cb in range(CT):
            xr = xrpool.tile([128, HALF], F32, name="xr", tag=f"xr{cb}")
            nc.sync.dma_start(out=xr, in_=xres[cb * 128:(cb + 1) * 128, :])
            xr_t.append(xr)

        # padded to free-step 16 bytes: dual-fp8 LDWEIGHTS requires the
        # interleave-pair step to be 16B-aligned
        ones2_full = consts.tile([128, 2, 16], FP8, name="ones2_full")
        nc.vector.memset(ones2_full, 1.0)
        ones2 = ones2_full[:, :, 0:1]

        def w_sl(nm, cs, ob):
            # lhsT [128, 2, 128] for output channel block ob
            return w2[nm][:, cs, :, ob * 128:(ob + 1) * 128]

        # --- fp8 DR work generators ---
        def k_half(jc, half):
            # K bias is dropped: softmax over j is invariant to the
            # j-independent term (q+bq)@bk.  Evac on DVE (plain copy).
            jsl = slice(jc * 512, (jc + 1) * 512)
            for ob in (2 * half, 2 * half + 1):
                ps = ps_kv.tile([128, 512], F32, name="ps_k", tag="kv")
                for cs in range(CS):
                    nc.tensor.matmul(ps, w_sl("k", cs, ob),
                                     xn2_t[cs][:, :, jsl],
                                     start=(cs == 0), stop=(cs == 1),
                                     perf_mode=DR)
                nc.vector.tensor_copy(out=k2_t[ob // 2][:, ob % 2, jsl],
                                      in_=ps)

        def q_group(ic, on_dve=False):
            # on_dve: kv-ring + DVE evac (engine-coherent) for chunks
            # where DVE idles but the ACT exp stream is pacing
            isl = slice(ic * 512, (ic + 1) * 512)
            for ob in range(CT):
                pool, tag = (ps_kv, "kv") if on_dve else (ps_st, "st")
                ps = pool.tile([128, 512], F32, name="ps_q", tag=tag)
                for cs in range(CS):
                    nc.tensor.matmul(ps, w_sl("q", cs, ob),
                                     xn2_t[cs][:, :, isl],
                                     start=(cs == 0), stop=(cs == 1),
                                     perf_mode=DR)
                if on_dve:
                    nc.vector.tensor_scalar(
                        out=q2_t[ob // 2][:, ob % 2, isl], in0=ps,
                        scalar1=bqs_t[ob], scalar2=None, op0=OP.add)
                else:
                    nc.scalar.activation(out=q2_t[ob // 2][:, ob % 2, isl],
                                         in_=ps, func=AF.Identity,
                                         bias=bqs_t[ob])

        def v_group(jb):
            # V bias folded into bo on the host (bo' = bo + wo@bv):
            # softmax weights sum to 1, so the bias passes through
            # attention unchanged -> evac is a plain copy
            ps = ps_kv.tile([128, 512], F32, name="ps_v", tag="kv")
            for cs in range(CS):
                nc.tensor.matmul(
                    ps, xn2_t[cs][:, :, jb * 128:(jb + 1) * 128],
                    w2["v"][:, cs, :, :],
                    start=(cs == 0), stop=(cs == 1), perf_mode=DR)
            nc.vector.tensor_copy(out=vt2_t[jb // 2][:, jb % 2, :], in_=ps)

        def st_group(ic, jb):
            ph = ic % 3
            ps = ps_st.tile([128, 512], F32, name="ps_stg", tag="st")
            for cs in range(CS):
                nc.tensor.matmul(
                    ps, k2_t[cs][:, :, jb * 128:(jb + 1) * 128],
                    q2_t[cs][:, :, ic * 512:(ic + 1) * 512],
                    start=(cs == 0), stop=(cs == 1), perf_mode=DR)
            # softmax scale (incl. the 1/64 weight-scale fold) in the exp
            nc.scalar.activation(out=pt_t[ph][jb // 2][:, jb % 2, :], in_=ps,
                                 func=AF.Exp, scale=SCALE / (WS * WS),
                                 bias=negs_t)

        chunk_state = {ic: {"sums": None, "oa": None, "o3": None,
                            "osb": None, "bcast": None}
                       for ic in range(IC)}

        def consume_a(ic, jp):
            # pass A: sums + O channel blocks 0-2
            st_c = chunk_state[ic]
            ph = ic % 3
            if st_c["sums"] is None:
                # last chunk's sums borrows the kv ring: the su bank is
                # still accumulating the PREVIOUS chunk's cb3 then
                pool = ps_kv if ic == IC - 1 else ps_su
                tag = "kv" if ic == IC - 1 else "su"
                st_c["sums"] = pool.tile([1, 512], F32, name="sums",
                                         tag=tag)
                st_c["oa"] = [ps_oa.tile([128, 512], F32, name="oa",
                                         tag=f"oa{cb}")
                              for cb in range(3)]
            nc.tensor.matmul(st_c["sums"], ones2, pt_t[ph][jp],
                             start=(jp == 0), stop=(jp == NP - 1),
                             skip_group_check=True, perf_mode=DR)
            for cb in range(3):
                nc.tensor.matmul(
                    st_c["oa"][cb], vt2_t[jp][:, :, cb * 128:(cb + 1) * 128],
                    pt_t[ph][jp], start=(jp == 0), stop=(jp == NP - 1),
                    skip_group_check=True, perf_mode=DR)

        def consume_b(ic, jp, pool=None, tag="su"):
            # pass B: O channel block 3 in the freed sums bank (or a
            # borrowed kv-ring bank when the su bank is still live)
            st_c = chunk_state[ic]
            ph = ic % 3
            if st_c["o3"] is None:
                st_c["o3"] = (pool or ps_su).tile([128, 512], F32,
                                                  name="o3", tag=tag)
            nc.tensor.matmul(
                st_c["o3"], vt2_t[jp][:, :, 384:512], pt_t[ph][jp],
                start=(jp == 0), stop=(jp == NP - 1),
                skip_group_check=True, perf_mode=DR)

        def evac_sums(ic):
            # reciprocal, then a Pool partition_broadcast (SBUF->SBUF,
            # ~0.8us) instead of the 2-DMA DRAM bounce; frees the su bank
            # for pass B
            st_c = chunk_state[ic]
            recip0 = finpool.tile([1, 512], F32, name="recip0", tag="recip0")
            nc.vector.reciprocal(out=recip0, in_=st_c["sums"])
            recip = finpool.tile([1, 512], F32, name="recip", tag="recip")
            nc.vector.tensor_scalar(out=recip, in0=recip0,
                                    scalar1=OS / (WS * WS), scalar2=None,
                                    op0=OP.mult)
            bcast = finpool.tile([128, 512], F32, name="bcast", tag="bcast")
            nc.gpsimd.partition_broadcast(bcast, recip)
            st_c["bcast"] = bcast

        def evac_oa(ic):
            # O (carrying the V-path x8) straight to fp8 channel pairs
            st_c = chunk_state[ic]
            st_c["osb"] = [opool.tile([128, 2, 512], FP8, name="o2",
                                      tag=f"o2_{cs}") for cs in range(CS)]
            for cb in range(3):
                nc.vector.tensor_scalar(
                    out=st_c["osb"][cb // 2][:, cb % 2, :],
                    in0=st_c["oa"][cb], scalar1=1.0 / OS, scalar2=None,
                    op0=OP.mult)

        def evac_o3(ic):
            st_c = chunk_state[ic]
            nc.vector.tensor_scalar(out=st_c["osb"][1][:, 1, :],
                                    in0=st_c["o3"], scalar1=1.0 / OS,
                                    scalar2=None, op0=OP.mult)

        def emit_y(ic, fin_eng):
            # fp8 DR out-conv (contraction = attention channels, pairs);
            # fin_eng: Pool mid-stream, DVE for the drain blocks where
            # Pool's 806ns ops would serialize the tail
            st_c = chunk_state[ic]
            o_sb_, bcast = st_c["osb"], st_c["bcast"]
            isl_ = slice(ic * 512, (ic + 1) * 512)
            for ob in range(CT):
                y_ps = ps_kv.tile([128, 512], F32, name="y_ps", tag="kv")
                for cs in range(CS):
                    nc.tensor.matmul(
                        y_ps, w_sl("o", cs, ob), o_sb_[cs],
                        start=(cs == 0), stop=(cs == 1), perf_mode=DR)
                t1 = finpool.tile([128, 512], F32, name="t1", tag="t1")
                nc.vector.tensor_mul(out=t1, in0=y_ps, in1=bcast)
                yf = finpool.tile([128, 512], F32, name="yf", tag="yf",
                                  bufs=4)
                if fin_eng == "pool":
                    yh = finpool.tile([128, 512], F32, name="yh", tag="yh")
                    nc.gpsimd.tensor_add(out=yh, in0=t1,
                                         in1=xr_t[ob][:, isl_])
                    nc.gpsimd.tensor_scalar(out=yf, in0=yh,
                                            scalar1=bo_t[ob],
                                            scalar2=None, op0=OP.add)
                else:
                    nc.vector.scalar_tensor_tensor(
                        out=yf, in0=t1, scalar=bo_t[ob],
                        in1=xr_t[ob][:, isl_], op0=OP.add, op1=OP.add)
                nc.scalar.dma_start(out=y[ob * 128:(ob + 1) * 128, isl_],
                                    in_=yf)

        # --- schedule ---
        # steady chunk ic: w<16: st + consume_a(ic-1); w16: recip+evac;
        # w17..31: st + consume_b(ic-1); next chunk w0 finishes pass B.
        # The y block of chunk ic-2 sits at w13.  Chunk 3 additionally
        # carries consume_a(3) lagged in its second half (its sums on the
        # kv ring since the su bank still runs chunk 2's pass B).
        k_half(0, 0)
        k_half(0, 1)
        k_half(1, 0)
        k_half(1, 1)
        q_group(0)
        vq = list(range(JB))             # V emission order
        for w in range(JB):              # ---- chunk 0 ----
            st_group(0, w)
            if w % 4 in (0, 1) and w // 4 + 2 < JC:
                k_half(w // 4 + 2, w % 4)
            elif w >= 26:
                v_group(vq.pop(0))
            if w == 24:
                q_group(1)
        for w in range(JB):              # ---- chunk 1 ----
            for _ in range(2):           # keep vt2[jp] >=2 windows ahead
                if vq:                   # of consume_a(0, jp)
                    v_group(vq.pop(0))
            st_group(1, w)
            if w < NP:
                consume_a(0, w)
            elif w == NP:
                evac_sums(0)
                evac_oa(0)
            if w > NP:
                consume_b(0, w - NP - 1)
            if w == 20:
                q_group(2)
        for w in range(JB):              # ---- chunk 2 ----
            if w == 0:
                consume_b(0, NP - 1)
                evac_o3(0)
            st_group(2, w)
            if w < NP:
                consume_a(1, w)
            elif w == NP:
                evac_sums(1)
                evac_oa(1)
            if w > NP:
                consume_b(1, w - NP - 1)
            if w == 13:
                emit_y(0, "pool")
            if w == 20:
                q_group(3, on_dve=True)
        for w in range(JB):              # ---- chunk 3 ----
            if w == 0:
                consume_b(1, NP - 1)
                evac_o3(1)
            st_group(3, w)
            if w < NP:
                consume_a(2, w)
                # chunk 2's pass B runs concurrently on a borrowed
                # kv-ring bank (c3 has no K/V work; y(1) only needs one
                # kv buf), so y(2) escapes the drain
                consume_b(2, w, pool=ps_kv, tag="kv")
            elif w == NP:
                evac_sums(2)
                evac_oa(2)
                evac_o3(2)
            if w > NP:
                consume_a(3, w - NP - 1)
            if w > NP + 1:
                consume_b(3, w - NP - 2)
            if w == 13:
                emit_y(1, "pool")
            if w == 20:
                emit_y(2, "pool")
        # drain: pass-B tail runs on PE while DVE drains the oa banks
        consume_a(3, NP - 1)
        evac_sums(3)
        for jp in range(NP - 2, NP):
            consume_b(3, jp)
        evac_oa(3)
        evac_o3(3)
        emit_y(3, "vector")

        ps_oa.release()
        ps_su.release()
        ps_kv.release()
        ps_st.release()
        finpool.release()
        opool.release()
        xrpool.release()
        xnpool.release()

        ptpool.release()
        qpool.release()
        vpool.release()
        kpool.release()
        wpool.release()
        consts.release()

    nc.compile()
    return nc


_cache = threading.Lock(), {}


def _get_nc():
    lock, d = _cache
    with lock:
        if "nc" not in d:
            d["nc"] = build_bass()
        return d["nc"]


def _pack_w2(w, scale=WS):
    """[Cout, Cin] float weight -> [128, cs*1024 + s*512 + o] fp8 where
    w2[p, cs, s, o] = w[o, cs*256 + s*128 + p] * scale."""
    wT = np.asarray(w, np.float32).T * scale          # [Cin, Cout]
    packed = wT.reshape(CS, 2, 128, C).transpose(2, 0, 1, 3).reshape(
        128, CS * 2 * C)
    return np.ascontiguousarray(packed).astype(ml_dtypes.float8_e4m3fn)


def kernel(x, gn_w, gn_b, wq, bq, wk, bk, wv, bv, wo, bo):
    x = np.asarray(x, dtype=np.float32)
    bf = ml_dtypes.bfloat16

    wq2 = _pack_w2(wq)
    wk2 = _pack_w2(wk)
    wv2 = _pack_w2(wv)
    wo2 = _pack_w2(wo)
    cols = np.stack([np.asarray(bq, np.float32) * WS,
                     np.asarray(bk, np.float32) * WS,
                     np.asarray(bo, np.float32)
                     + np.asarray(wo, np.float32) @ np.asarray(bv, np.float32),
                     np.asarray(gn_w, np.float32),
                     np.asarray(gn_b, np.float32)], axis=1)  # [C, 5]
    colb = np.ascontiguousarray(
        cols.reshape(CT, 128, 5).transpose(1, 0, 2).reshape(128, CT * 5))
    # block-diagonal group-mean map: 8 groups of 16 channels per 128-tile
    gmap = (np.kron(np.eye(8, dtype=np.float32),
                    np.ones((16, 16), np.float32)) / 16.0)

    xr = x.reshape(B, C, HW)
    in_maps = []
    for core in range(NCORES):
        b, h = divmod(core, 2)
        xs = xr[b]
        if h:
            xs = np.concatenate([xs[:, HALF:], xs[:, :HALF]], axis=1)
        in_maps.append({
            "xbf": np.ascontiguousarray(xs).astype(bf),
            "xres": np.ascontiguousarray(xs[:, :HALF]),
            "wq2": wq2, "wk2": wk2, "wv2": wv2, "wo2": wo2,
            "colb": colb, "gmap": gmap,
        })

    from concourse.bass_utils import run_bass_kernel_spmd
    nc = _get_nc()
    res = run_bass_kernel_spmd(nc, in_maps, core_ids=list(range(NCORES)))

    out = np.empty((B, C, HW), np.float32)
    for core in range(NCORES):
        b, h = divmod(core, 2)
        out[b][:, h * HALF:(h + 1) * HALF] = res.results[core]["y"]
    return out.reshape(B, C, H, W)
